# revision 43
# baseline (speedup 1.0000x reference)
"""Trainium2 Bass kernel: GroupNorm(8) -> 1x1 QKV conv -> 4-head attention
(n=4096, dim_head=32) -> 1x1 out conv, for x[4, 256, 64, 64] f32.

Sharding (8 cores, SPMD, no collectives): core c handles batch c//2 and query
half c%2. Each core receives the full 256x4096 (channels x spatial) slab of
its batch -- spatially ROLLED by 2048 for odd cores, so the program always
computes attention outputs for "queries 0:2048". GroupNorm statistics and
attention are invariant under a permutation of the key/spatial axis, so the
rolled copy yields exactly the outputs of the core's query half.

Device-side layout highlights:
  - sim is computed transposed, [keys, queries], per head at PE row-strip 32h
    (head pairs run concurrently via tile_position row tiling), so softmax's
    denominator folds into the attn@V matmul as an appended ones-column
    (stationary V_ext [128 keys, 33]) -> psum rows 0:32 hold O^T [dh, q]
    (exactly the layout the out-projection wants), row 32 the denominator.
  - exp runs on ScalarE from PSUM f32 -> SBUF bf16 in [128, 3x512] chunks
    (6 of the 8 psum banks double-buffer the sim tiles; 1 bank accumulates
    O^T; 1 bank serves projections/broadcasts). ScalarE is the wall-clock
    bottleneck: ~33.5M exps per core at ~1 elem/cycle/lane.
  - all matmuls are bf16 (fp32 matmul is 4x slower on TRN2); x ships as bf16
    to halve the startup HBM load; everything hidden under exp except the
    ~15us in-ramp and ~11us drain tail (cost-model timeline ~298us/core).
  - K/Q/V_T and the remaining GroupNorm-ed x chunks are produced inside the
    first attention segment, borrowing idle engine time under the exp stream;
    per-(qb,pair) epilogues (reciprocal + rank-1 broadcast matmul + scale,
    out-projection) are deferred past the next segment's first chunks so the
    PE keeps feeding ScalarE.
"""

import ml_dtypes
import numpy as np

HEADS, DH, G, EPS = 4, 32, 8, 1e-5
B, C, HW = 4, 256, 64
N = HW * HW          # 4096 spatial positions (keys)
NQ = N // 2          # 2048 queries per core
NKB = N // 128       # 32 key blocks
NQB = NQ // 512      # 4 query blocks of 512
CT = C // 128        # 2 channel tiles
HID = HEADS * DH     # 128

_BUILT = {}


_MAX_INST_WAITS = 1


def _patch_tail_drain(tile_mod):
    """Walrus codegen on this toolchain only supports a small number of sync
    waits per ISA instruction. Two patches:
    1. every committed instruction with too many waits gets the excess hoisted
       onto same-engine nops emitted immediately before it (same stream
       position => identical semantics);
    2. the TileContext tail drain (one wait per engine + DMA lane, >8 total)
       is split one wait per SP nop."""
    if getattr(tile_mod.TileContext, "_drain_patched", False):
        return
    import bass_rust
    from concourse.vector_clock import ScopedClock

    _orig_add_instruction = tile_mod.TileContext._add_instruction

    _SELF_WAIT_OK = ("InstActivation",)

    def _add_with_wait_split(self, inst):
        si = getattr(inst, "sync_info", None)
        if (
            si is not None
            and si.on_wait
            and type(inst).__name__ in _SELF_WAIT_OK
        ):
            # engine queues are strict FIFO: a data op's wait on its own
            # engine's sem is redundant (prior same-engine ops complete in
            # order before it) -- dropping it avoids a split nop per op
            eng_name = str(inst.engine).split(".")[-1]
            kept = [
                w for w in si.on_wait
                if w.ant_name.rsplit("_", 1)[0] != eng_name
            ]
            if len(kept) != len(si.on_wait):
                import bass_rust as _br
                inst.sync_info = _br.SyncInfo(
                    on_wait=kept, on_update=list(si.on_update)
                )
                si = inst.sync_info
        if si is not None and len(si.on_wait) > _MAX_INST_WAITS:
            waits = list(si.on_wait)
            keep, excess = waits[: _MAX_INST_WAITS], waits[_MAX_INST_WAITS :]
            eng = self.nc.engines[inst.engine]
            for w in excess:  # NoOps only support a single wait slot
                nop = eng.nop(nofuse=True, hint="wait_split")
                nop.ins.sync_info = bass_rust.SyncInfo(on_wait=[w], on_update=[])
            inst.sync_info = bass_rust.SyncInfo(
                on_wait=keep, on_update=list(si.on_update)
            )
        return _orig_add_instruction(self, inst)

    tile_mod.TileContext._add_instruction = _add_with_wait_split

    def _patched(self, tick_clock, wait_clock):
        nop = self.nc.sync.nop(nofuse=True, hint="pre_drain_wait_split")
        wait_clock.add_sem_waits(nop.ins, ScopedClock({None: tick_clock.global_clock}))
        si = nop.ins.sync_info
        waits = list(si.on_wait) if si is not None else []
        if len(waits) > 1:
            nop.ins.sync_info = bass_rust.SyncInfo(
                on_wait=waits[:1], on_update=list(si.on_update)
            )
            for i in range(1, len(waits)):
                n2 = self.nc.sync.nop(nofuse=True, hint="pre_drain_wait_split")
                n2.ins.sync_info = bass_rust.SyncInfo(on_wait=[waits[i]], on_update=[])
        self.nc.sync.drain()
        self.nc.all_engine_barrier()
        assert self.sems is not None
        popped = self.nc._tile_sem_poison_stack.pop()
        assert popped is self._sem_poison
        self.nc.clear_and_free_semaphores(list(self.sems.allocated().values()))
        self.nc.all_engine_barrier()

    tile_mod.TileContext._drain_and_barrier = _patched
    tile_mod.TileContext._drain_patched = True


def build_nc():
    import concourse.bass as bass
    import concourse.mybir as mybir
    import concourse.tile as tile

    _patch_tail_drain(tile)
    f32 = mybir.dt.float32
    bf16 = mybir.dt.bfloat16
    AF = mybir.ActivationFunctionType
    ALU = mybir.AluOpType

    nc = bass.Bass()
    x_d = nc.declare_dram_parameter("x", [CT, 128, N], bf16, isOutput=False)
    wq_d = nc.declare_dram_parameter("wq", [CT, 128, 3 * HID], f32, isOutput=False)
    wo_d = nc.declare_dram_parameter("wo", [HID, C], f32, isOutput=False)
    gb_d = nc.declare_dram_parameter("gb", [128, CT, 3], f32, isOutput=False)
    gmask_d = nc.declare_dram_parameter("gmask", [128, CT, G], f32, isOutput=False)
    sel_d = nc.declare_dram_parameter("selT", [G, CT, 128], f32, isOutput=False)
    out_d = nc.declare_dram_parameter("out", [CT, 128, NQ], bf16, isOutput=True)

    with tile.TileContext(nc) as tc:
        with (
            tc.tile_pool(name="big", bufs=1) as big,
            tc.tile_pool(name="work", bufs=2) as work,
            tc.tile_pool(name="epool", bufs=3) as epool,
            tc.tile_pool(name="psS", bufs=2, space="PSUM") as psS,  # 2 x 3 banks
            tc.tile_pool(name="psO", bufs=1, space="PSUM") as psO,  # 1 bank
            tc.tile_pool(name="psM", bufs=1, space="PSUM") as psM,  # 1 bank
        ):
            # ---------------- load inputs ----------------
            x_sb = big.tile([128, CT, N], bf16)
            for t in range(CT):
                for j in range(4):
                    nc.sync.dma_start(
                        x_sb[:, t, j * 1024 : (j + 1) * 1024],
                        x_d[t, :, j * 1024 : (j + 1) * 1024],
                    )
            wq_sb = big.tile([128, CT, 3 * HID], f32)
            nc.sync.dma_start(wq_sb[:, 0], wq_d[0])
            nc.sync.dma_start(wq_sb[:, 1], wq_d[1])
            wo_sb = big.tile([HID, C], f32)
            nc.sync.dma_start(wo_sb[:], wo_d[:])
            gb_sb = big.tile([128, CT, 3], f32)
            nc.sync.dma_start(gb_sb[:], gb_d[:])
            gmask_sb = big.tile([128, CT, G], f32)
            nc.sync.dma_start(gmask_sb[:], gmask_d[:])
            sel_sb = big.tile([G, CT, 128], f32)
            nc.sync.dma_start(sel_sb[:], sel_d[:])

            wq_bf = big.tile([128, CT, 3 * HID], bf16)
            nc.vector.tensor_copy(wq_bf[:], wq_sb[:])
            wo_bf = big.tile([HID, C], bf16)
            nc.vector.tensor_copy(wo_bf[:], wo_sb[:])

            # ---------------- group norm ----------------
            # per-channel mean/var via DVE bn_stats (512-col chunks, overlaps
            # the x DMAs); T = [mean_c, E[x^2]_c]
            NJ = 4
            CHK = N // NJ
            T_sb = work.tile([128, CT, 2], f32, tag="gnT")
            for t in range(CT):
                bst = work.tile([128, 8, nc.vector.BN_STATS_DIM], f32, tag="gnbst")
                for j in range(8):
                    nc.vector.bn_stats(
                        out=bst[:, j, :], in_=x_sb[:, t, j * 512 : (j + 1) * 512]
                    )
                mv = work.tile([128, nc.vector.BN_AGGR_DIM], f32, tag="gnmv")
                nc.vector.bn_aggr(out=mv[:], in_=bst[:])
                # T[:,t] = [mean, var + mean^2]
                nc.vector.tensor_copy(T_sb[:, t, 0:1], mv[:, 0:1])
                nc.vector.tensor_mul(T_sb[:, t, 1:2], mv[:, 0:1], mv[:, 0:1])
                nc.vector.tensor_tensor(
                    T_sb[:, t, 1:2], T_sb[:, t, 1:2], mv[:, 1:2], ALU.add
                )
            # group stats: [G, 2] = sum over channels-in-group / (32*4096)
            stats_ps = psM.tile([G, 2], f32, tag="misc")
            for t in range(CT):
                nc.tensor.matmul(
                    stats_ps[:], gmask_sb[:, t, :], T_sb[:, t, :],
                    start=(t == 0), stop=(t == CT - 1),
                )
            # var = E[x^2] - mean^2 ; rstd = 1/sqrt(var+eps)
            stats_sb = work.tile([G, 2], f32, tag="gnstats")  # [mean, rstd]
            stats_tmp = work.tile([G, 2], f32, tag="gnstats_raw")
            msq = work.tile([G, 1], f32, tag="gnmsq")
            nc.vector.tensor_copy(stats_tmp[:], stats_ps[:])
            nc.vector.tensor_copy(stats_sb[:, 0:1], stats_tmp[:, 0:1])
            nc.vector.tensor_mul(msq[:], stats_tmp[:, 0:1], stats_tmp[:, 0:1])
            nc.vector.tensor_tensor(
                stats_sb[:, 1:2], stats_tmp[:, 1:2], msq[:], ALU.subtract
            )
            eps_sb = work.tile([G, 1], f32, tag="gneps")
            nc.vector.memset(eps_sb[:], EPS)
            nc.scalar.activation(
                stats_sb[:, 1:2], stats_sb[:, 1:2], AF.Sqrt, bias=eps_sb[:]
            )
            nc.vector.reciprocal(stats_sb[:, 1:2], stats_sb[:, 1:2])
            # broadcast to channels, fold gamma/beta, normalize -> xn (bf16)
            xn_bf = big.tile([128, CT, N], bf16)
            scs = []
            bc_ps = psM.tile([128, CT, 2], f32, tag="misc")  # [mean_c, rstd_c] per t
            for t in range(CT):
                nc.tensor.matmul(
                    bc_ps[:, t, :], sel_sb[:, t, :], stats_sb[:], start=True, stop=True
                )
            for t in range(CT):
                sc = work.tile([128, 2], f32, tag=f"gnsc{t}")  # [scale_c, bias_c]
                nc.vector.tensor_mul(sc[:, 0:1], gb_sb[:, t, 0:1], bc_ps[:, t, 1:2])
                nc.vector.tensor_mul(sc[:, 1:2], bc_ps[:, t, 0:1], sc[:, 0:1])
                nc.vector.tensor_tensor(
                    sc[:, 1:2], gb_sb[:, t, 1:2], sc[:, 1:2], ALU.subtract
                )
                scs.append(sc)

            # ---------------- QKV projections ----------------
            # Only K block 0 and Q block 0 are produced up front; the rest is
            # interleaved into the first attention segment so ScalarE starts
            # exp-ing as early as possible.
            k_bf = big.tile([128, N], bf16)      # [hid, keys] head-major
            q_bf = big.tile([128, NQ], bf16)     # [hid, queries] (pre-scaled W)
            v_bf = big.tile([128, NKB, 136], bf16)  # [key128, kb, 4x(32 V + 1 ones) + pad]

            def emit_k(cb):
                kp = psM.tile([128, 512], f32, tag="misc")
                for t in range(CT):
                    nc.tensor.matmul(
                        kp[:], wq_bf[:, t, HID : 2 * HID],
                        xn_bf[:, t, cb * 512 : (cb + 1) * 512],
                        start=(t == 0), stop=(t == CT - 1),
                    )
                if cb == 0:  # ScalarE is idle pre-attention; keep DVE free
                    nc.scalar.activation(
                        k_bf[:, 0:512], kp[:], AF.Identity
                    )
                else:
                    nc.vector.tensor_copy(k_bf[:, cb * 512 : (cb + 1) * 512], kp[:])

            def emit_q(cb):
                qp = psM.tile([128, 512], f32, tag="misc")
                for t in range(CT):
                    nc.tensor.matmul(
                        qp[:], wq_bf[:, t, 0:HID],
                        xn_bf[:, t, cb * 512 : (cb + 1) * 512],
                        start=(t == 0), stop=(t == CT - 1),
                    )
                if cb == 0:
                    nc.scalar.activation(
                        q_bf[:, 0:512], qp[:], AF.Identity
                    )
                else:
                    nc.vector.tensor_copy(q_bf[:, cb * 512 : (cb + 1) * 512], qp[:])

            def emit_v(nb):
                vp = psM.tile([128, 128], f32, tag="misc")
                for t in range(CT):
                    nc.tensor.matmul(
                        vp[:], xn_bf[:, t, nb * 128 : (nb + 1) * 128],
                        wq_bf[:, t, 2 * HID : 3 * HID],
                        start=(t == 0), stop=(t == CT - 1),
                    )
                nc.vector.tensor_copy(
                    v_bf[:, nb, 0:132].rearrange("p (h c) -> p h c", c=33)[:, :, 0:32],
                    vp.rearrange("p (h c) -> p h c", c=32),
                )

            nc.vector.memset(
                v_bf[:, :, 0:132].rearrange("p k (h c) -> p k h c", c=33)[:, :, :, 32:33],
                1.0,
            )
            def emit_xn(j):
                c_sl = slice(j * CHK, (j + 1) * CHK)
                for t in range(CT):
                    nc.vector.tensor_scalar(
                        xn_bf[:, t, c_sl], x_sb[:, t, c_sl],
                        scalar1=scs[t][:, 0:1], scalar2=scs[t][:, 1:2],
                        op0=ALU.mult, op1=ALU.add,
                    )

            emit_xn(0)
            emit_k(0)
            emit_q(0)

            # ---------------- attention + out-projection ----------------
            h_bf = big.tile([128, NQ], bf16)     # [hid, queries], normalized
            ones32 = big.tile([1, 32], bf16)
            nc.vector.memset(ones32[:], 1.0)

            def make_normalize(qb, pair, oacc):
                # epilogue for one (qb, pair): 1/denominator, rank-1 bcast to
                # 32 partitions (bf16 matmul), scale O^T -> h_bf
                def _emit():
                    q_sl = slice(qb * 512, (qb + 1) * 512)
                    # one copy releases the psO slot quickly; the epilogue
                    # then works from SBUF
                    ocp = work.tile([128, 512], f32, tag="ocp")
                    nc.vector.tensor_copy(ocp[:], oacc[:])
                    for hi in range(2):
                        h = 2 * pair + hi
                        r = work.tile([1, 512], bf16, tag="r")
                        with nc.allow_low_precision(
                            reason="softmax denom reciprocal at bf16 matches matmul dtype"
                        ):
                            nc.vector.reciprocal(
                                r[:], ocp[64 * hi + 32 : 64 * hi + 33, :]
                            )
                        rb = psM.tile([32, 512], f32, tag="misc")
                        nc.tensor.matmul(rb[:], ones32[:], r[:], start=True, stop=True)
                        rbs = work.tile([128, 512], f32, tag="rbs")
                        nc.vector.tensor_copy(rbs[64 * hi : 64 * hi + 32, :], rb[:])
                        nc.vector.tensor_tensor(
                            h_bf[32 * h : 32 * h + 32, q_sl],
                            ocp[64 * hi : 64 * hi + 32, :],
                            rbs[64 * hi : 64 * hi + 32, :],
                            ALU.mult,
                        )
                return _emit

            def make_outproj(qb, use_act=False):
                def _emit():
                    q_sl = slice(qb * 512, (qb + 1) * 512)
                    for oc in range(CT):
                        yp = psM.tile([128, 512], f32, tag="misc")
                        nc.tensor.matmul(
                            yp[:], wo_bf[:, oc * 128 : (oc + 1) * 128], h_bf[:, q_sl],
                            start=True, stop=True,
                        )
                        yt = work.tile([128, 512], bf16, tag="yt")
                        if use_act:  # tail: ScalarE is idle after the last exp
                            nc.scalar.activation(
                                yt[:], yp[:], AF.Identity, bias=gb_sb[:, oc, 2:3]
                            )
                        else:
                            nc.vector.tensor_scalar(
                                yt[:], yp[:], scalar1=gb_sb[:, oc, 2:3], scalar2=None,
                                op0=ALU.add,
                            )
                        nc.sync.dma_start(out_d[oc, :, q_sl], yt[:])
                return _emit

            # per-segment slice index i in [0, 64): kb = i//2, head = pair*2 + i%2.
            # ScalarE exp chunks cover 3 slices (1536 elems/partition) to
            # amortize the per-ACTIVATE overhead; last two chunks are 2-wide.
            CH_SIZES = [2, 3, 3, 3, 3, 3, 3, 3, 3, 3, 3, 3, 3, 3, 3, 3, 3, 3, 3, 3, 3, 2]
            pending = []  # epilogues deferred so PE keeps feeding ACT first
            for qb in range(NQB):
                q_sl = slice(qb * 512, (qb + 1) * 512)
                for pair in range(2):
                    seg0 = qb == 0 and pair == 0
                    if pending:
                        for fn in pending:
                            fn()
                        pending = []
                    oacc = psO.tile([128, 512], f32, tag="oacc")
                    i = 0
                    for ci, ch in enumerate(CH_SIZES):
                        simp = psS.tile([128, ch, 512], f32, tag="sim")
                        for s in range(ch):
                            kb, hi = (i + s) // 2, (i + s) % 2
                            h = 2 * pair + hi
                            nc.tensor.matmul(
                                simp[:, s, :],
                                k_bf[32 * h : 32 * h + 32, kb * 128 : (kb + 1) * 128],
                                q_bf[32 * h : 32 * h + 32, q_sl],
                                start=True, stop=True,
                                tile_position=(32 * h, 0),
                            )
                        if seg0:
                            # produce the remaining xn chunks, V_T for this
                            # chunk's key blocks, and the next K column block,
                            # overlapped with the exp
                            if ci in (1, 6, 11):
                                emit_xn({1: 1, 6: 2, 11: 3}[ci])
                            for kb in range((i + 1) // 2, (i + ch + 1) // 2):
                                emit_v(kb)
                                if kb % 4 == 0 and 0 < kb // 4 + 1 < 8:
                                    emit_k(kb // 4 + 1)
                        if qb < NQB - 1 and pair == 1 and ci == 8:
                            emit_q(qb + 1)
                        e = epool.tile([128, ch, 512], bf16, tag="e")
                        nc.scalar.activation(e[:], simp[:], AF.Exp)
                        for s in range(ch):
                            kb, hi = (i + s) // 2, (i + s) % 2
                            h = 2 * pair + hi
                            nc.tensor.matmul(
                                oacc[64 * hi : 64 * hi + 33, :],
                                v_bf[:, kb, 33 * h : 33 * h + 33],
                                e[:, s, :],
                                start=(kb == 0), stop=(kb == NKB - 1),
                                tile_position=(0, 64 * hi),
                            )
                        i += ch
                    pending.append(make_normalize(qb, pair, oacc))
                    if pair == 1:
                        pending.append(make_outproj(qb, use_act=(qb == NQB - 1)))
            for fn in pending:
                fn()
    return nc


def _prep_shared(w_qkv, w_out, b_out, gamma, beta):
    scale = DH ** -0.5
    wqkvT = np.ascontiguousarray(w_qkv.T).astype(np.float32).copy()  # [C, 384]
    wqkvT[:, :HID] *= scale
    wq = np.ascontiguousarray(wqkvT.reshape(CT, 128, 3 * HID))
    wo = np.ascontiguousarray(w_out.T).astype(np.float32)            # [HID, C]
    gb = np.stack(
        [
            np.asarray(gamma, np.float32).reshape(CT, 128).T,
            np.asarray(beta, np.float32).reshape(CT, 128).T,
            np.asarray(b_out, np.float32).reshape(CT, 128).T,
        ],
        axis=-1,
    )  # [128, CT, 3]
    gmask = np.zeros((128, CT, G), np.float32)
    sel = np.zeros((G, CT, 128), np.float32)
    for t in range(CT):
        for p in range(128):
            g = (t * 128 + p) // (C // G)
            gmask[p, t, g] = 1.0 / (C // G)
            sel[g, t, p] = 1.0
    return wq, wo, gb, gmask, sel


def _run(inputs, trace=False):
    from concourse.bass_utils import run_bass_kernel_spmd

    x = np.asarray(inputs["x"], np.float32)
    wq, wo, gb, gmask, sel = _prep_shared(
        np.asarray(inputs["w_qkv"], np.float32),
        np.asarray(inputs["w_out"], np.float32),
        np.asarray(inputs["b_out"], np.float32),
        np.asarray(inputs["gamma"], np.float32),
        np.asarray(inputs["beta"], np.float32),
    )
    if "nc" not in _BUILT:
        _BUILT["nc"] = build_nc()
    nc = _BUILT["nc"]

    in_maps = []
    for core in range(8):
        b_idx, qh = core // 2, core % 2
        xb = x[b_idx].reshape(C, N)
        if qh:
            xb = np.roll(xb, -NQ, axis=1)
        in_maps.append(
            {
                "x": np.ascontiguousarray(
                    xb.reshape(CT, 128, N).astype(ml_dtypes.bfloat16)
                ),
                "wq": wq, "wo": wo, "gb": gb, "gmask": gmask, "selT": sel,
            }
        )
    res = run_bass_kernel_spmd(
        nc, in_maps, core_ids=list(range(8)), trace=trace
    )
    out = np.empty((B, C, N), np.float32)
    for core in range(8):
        b_idx, qh = core // 2, core % 2
        y = res.results[core]["out"].astype(np.float32).reshape(C, NQ)
        out[b_idx, :, qh * NQ : (qh + 1) * NQ] = y
    return out.reshape(B, C, HW, HW), res


def kernel(**inputs) -> np.ndarray:
    out, _ = _run(inputs, trace=False)
    return out


# revision 45
# speedup vs baseline: 1.0024x; 1.0024x over previous
"""Trainium2 Bass kernel: GroupNorm(8) -> 1x1 QKV conv -> 4-head attention
(n=4096, dim_head=32) -> 1x1 out conv, for x[4, 256, 64, 64] f32.

Sharding (8 cores, SPMD, no collectives): core c handles batch c//2 and query
half c%2. Each core receives the full 256x4096 (channels x spatial) slab of
its batch -- spatially ROLLED by 2048 for odd cores, so the program always
computes attention outputs for "queries 0:2048". GroupNorm statistics and
attention are invariant under a permutation of the key/spatial axis, so the
rolled copy yields exactly the outputs of the core's query half.

Device-side layout highlights:
  - sim is computed transposed, [keys, queries], per head at PE row-strip 32h
    (head pairs run concurrently via tile_position row tiling), so softmax's
    denominator folds into the attn@V matmul as an appended ones-column
    (stationary V_ext [128 keys, 33]) -> psum rows 0:32 hold O^T [dh, q]
    (exactly the layout the out-projection wants), row 32 the denominator.
  - exp runs on ScalarE from PSUM f32 -> SBUF bf16 in [128, 3x512] chunks
    (6 of the 8 psum banks double-buffer the sim tiles; 1 bank accumulates
    O^T; 1 bank serves projections/broadcasts). ScalarE is the wall-clock
    bottleneck: ~33.5M exps per core at ~1 elem/cycle/lane.
  - all matmuls are bf16 (fp32 matmul is 4x slower on TRN2); x ships as bf16
    to halve the startup HBM load; everything hidden under exp except the
    ~15us in-ramp and ~11us drain tail (cost-model timeline ~298us/core).
  - K/Q/V_T and the remaining GroupNorm-ed x chunks are produced inside the
    first attention segment, borrowing idle engine time under the exp stream;
    per-(qb,pair) epilogues (reciprocal + rank-1 broadcast matmul + scale,
    out-projection) are deferred past the next segment's first chunks so the
    PE keeps feeding ScalarE.
"""

import ml_dtypes
import numpy as np

HEADS, DH, G, EPS = 4, 32, 8, 1e-5
B, C, HW = 4, 256, 64
N = HW * HW          # 4096 spatial positions (keys)
NQ = N // 2          # 2048 queries per core
NKB = N // 128       # 32 key blocks
NQB = NQ // 512      # 4 query blocks of 512
CT = C // 128        # 2 channel tiles
HID = HEADS * DH     # 128

_BUILT = {}


_MAX_INST_WAITS = 1


def _patch_tail_drain(tile_mod):
    """Walrus codegen on this toolchain only supports a small number of sync
    waits per ISA instruction. Two patches:
    1. every committed instruction with too many waits gets the excess hoisted
       onto same-engine nops emitted immediately before it (same stream
       position => identical semantics);
    2. the TileContext tail drain (one wait per engine + DMA lane, >8 total)
       is split one wait per SP nop."""
    if getattr(tile_mod.TileContext, "_drain_patched", False):
        return
    import bass_rust
    from concourse.vector_clock import ScopedClock

    _orig_add_instruction = tile_mod.TileContext._add_instruction

    _SELF_WAIT_OK = ("InstActivation",)

    def _add_with_wait_split(self, inst):
        si = getattr(inst, "sync_info", None)
        if (
            si is not None
            and si.on_wait
            and type(inst).__name__ in _SELF_WAIT_OK
        ):
            # engine queues are strict FIFO: a data op's wait on its own
            # engine's sem is redundant (prior same-engine ops complete in
            # order before it) -- dropping it avoids a split nop per op
            eng_name = str(inst.engine).split(".")[-1]
            kept = [
                w for w in si.on_wait
                if w.ant_name.rsplit("_", 1)[0] != eng_name
            ]
            if len(kept) != len(si.on_wait):
                import bass_rust as _br
                inst.sync_info = _br.SyncInfo(
                    on_wait=kept, on_update=list(si.on_update)
                )
                si = inst.sync_info
        if si is not None and len(si.on_wait) > _MAX_INST_WAITS:
            waits = list(si.on_wait)
            keep, excess = waits[: _MAX_INST_WAITS], waits[_MAX_INST_WAITS :]
            eng = self.nc.engines[inst.engine]
            for w in excess:  # NoOps only support a single wait slot
                nop = eng.nop(nofuse=True, hint="wait_split")
                nop.ins.sync_info = bass_rust.SyncInfo(on_wait=[w], on_update=[])
            inst.sync_info = bass_rust.SyncInfo(
                on_wait=keep, on_update=list(si.on_update)
            )
        return _orig_add_instruction(self, inst)

    tile_mod.TileContext._add_instruction = _add_with_wait_split

    def _patched(self, tick_clock, wait_clock):
        nop = self.nc.sync.nop(nofuse=True, hint="pre_drain_wait_split")
        wait_clock.add_sem_waits(nop.ins, ScopedClock({None: tick_clock.global_clock}))
        si = nop.ins.sync_info
        waits = list(si.on_wait) if si is not None else []
        if len(waits) > 1:
            nop.ins.sync_info = bass_rust.SyncInfo(
                on_wait=waits[:1], on_update=list(si.on_update)
            )
            for i in range(1, len(waits)):
                n2 = self.nc.sync.nop(nofuse=True, hint="pre_drain_wait_split")
                n2.ins.sync_info = bass_rust.SyncInfo(on_wait=[waits[i]], on_update=[])
        self.nc.sync.drain()
        self.nc.all_engine_barrier()
        assert self.sems is not None
        popped = self.nc._tile_sem_poison_stack.pop()
        assert popped is self._sem_poison
        self.nc.clear_and_free_semaphores(list(self.sems.allocated().values()))
        self.nc.all_engine_barrier()

    tile_mod.TileContext._drain_and_barrier = _patched
    tile_mod.TileContext._drain_patched = True


def build_nc():
    import concourse.bass as bass
    import concourse.mybir as mybir
    import concourse.tile as tile

    _patch_tail_drain(tile)
    f32 = mybir.dt.float32
    bf16 = mybir.dt.bfloat16
    AF = mybir.ActivationFunctionType
    ALU = mybir.AluOpType

    nc = bass.Bass()
    x_d = nc.declare_dram_parameter("x", [CT, 128, N], bf16, isOutput=False)
    wq_d = nc.declare_dram_parameter("wq", [CT, 128, 3 * HID], f32, isOutput=False)
    wo_d = nc.declare_dram_parameter("wo", [HID, C], f32, isOutput=False)
    gb_d = nc.declare_dram_parameter("gb", [128, CT, 3], f32, isOutput=False)
    gmask_d = nc.declare_dram_parameter("gmask", [128, CT, G], f32, isOutput=False)
    sel_d = nc.declare_dram_parameter("selT", [G, CT, 128], f32, isOutput=False)
    out_d = nc.declare_dram_parameter("out", [CT, 128, NQ], bf16, isOutput=True)

    with tile.TileContext(nc) as tc:
        with (
            tc.tile_pool(name="big", bufs=1) as big,
            tc.tile_pool(name="work", bufs=2) as work,
            tc.tile_pool(name="epool", bufs=3) as epool,
            tc.tile_pool(name="psS", bufs=2, space="PSUM") as psS,  # 2 x 3 banks
            tc.tile_pool(name="psO", bufs=1, space="PSUM") as psO,  # 1 bank
            tc.tile_pool(name="psM", bufs=1, space="PSUM") as psM,  # 1 bank
        ):
            # ---------------- load inputs ----------------
            x_sb = big.tile([128, CT, N], bf16)
            for t in range(CT):
                for j in range(4):
                    nc.sync.dma_start(
                        x_sb[:, t, j * 1024 : (j + 1) * 1024],
                        x_d[t, :, j * 1024 : (j + 1) * 1024],
                    )
            wq_sb = big.tile([128, CT, 3 * HID], f32)
            nc.sync.dma_start(wq_sb[:, 0], wq_d[0])
            nc.sync.dma_start(wq_sb[:, 1], wq_d[1])
            wo_sb = big.tile([HID, C], f32)
            nc.sync.dma_start(wo_sb[:], wo_d[:])
            gb_sb = big.tile([128, CT, 3], f32)
            nc.sync.dma_start(gb_sb[:], gb_d[:])
            gmask_sb = big.tile([128, CT, G], f32)
            nc.sync.dma_start(gmask_sb[:], gmask_d[:])
            sel_sb = big.tile([G, CT, 128], f32)
            nc.sync.dma_start(sel_sb[:], sel_d[:])

            wq_bf = big.tile([128, CT, 3 * HID], bf16)
            nc.vector.tensor_copy(wq_bf[:], wq_sb[:])
            wo_bf = big.tile([HID, C], bf16)
            nc.vector.tensor_copy(wo_bf[:], wo_sb[:])

            # ---------------- group norm ----------------
            # per-channel mean/var via DVE bn_stats (512-col chunks, overlaps
            # the x DMAs); T = [mean_c, E[x^2]_c]
            NJ = 4
            CHK = N // NJ
            T_sb = work.tile([128, CT, 2], f32, tag="gnT")
            for t in range(CT):
                bst = work.tile([128, 8, nc.vector.BN_STATS_DIM], f32, tag="gnbst")
                for j in range(8):
                    nc.vector.bn_stats(
                        out=bst[:, j, :], in_=x_sb[:, t, j * 512 : (j + 1) * 512]
                    )
                mv = work.tile([128, nc.vector.BN_AGGR_DIM], f32, tag="gnmv")
                nc.vector.bn_aggr(out=mv[:], in_=bst[:])
                # T[:,t] = [mean, var + mean^2]
                nc.vector.tensor_copy(T_sb[:, t, 0:1], mv[:, 0:1])
                nc.vector.tensor_mul(T_sb[:, t, 1:2], mv[:, 0:1], mv[:, 0:1])
                nc.vector.tensor_tensor(
                    T_sb[:, t, 1:2], T_sb[:, t, 1:2], mv[:, 1:2], ALU.add
                )
            # group stats: [G, 2] = sum over channels-in-group / (32*4096)
            stats_ps = psM.tile([G, 2], f32, tag="misc")
            for t in range(CT):
                nc.tensor.matmul(
                    stats_ps[:], gmask_sb[:, t, :], T_sb[:, t, :],
                    start=(t == 0), stop=(t == CT - 1),
                )
            # var = E[x^2] - mean^2 ; rstd = 1/sqrt(var+eps)
            stats_sb = work.tile([G, 2], f32, tag="gnstats")  # [mean, rstd]
            stats_tmp = work.tile([G, 2], f32, tag="gnstats_raw")
            msq = work.tile([G, 1], f32, tag="gnmsq")
            nc.vector.tensor_copy(stats_tmp[:], stats_ps[:])
            nc.vector.tensor_copy(stats_sb[:, 0:1], stats_tmp[:, 0:1])
            nc.vector.tensor_mul(msq[:], stats_tmp[:, 0:1], stats_tmp[:, 0:1])
            nc.vector.tensor_tensor(
                stats_sb[:, 1:2], stats_tmp[:, 1:2], msq[:], ALU.subtract
            )
            eps_sb = work.tile([G, 1], f32, tag="gneps")
            nc.vector.memset(eps_sb[:], EPS)
            nc.scalar.activation(
                stats_sb[:, 1:2], stats_sb[:, 1:2], AF.Sqrt, bias=eps_sb[:]
            )
            nc.vector.reciprocal(stats_sb[:, 1:2], stats_sb[:, 1:2])
            # broadcast to channels, fold gamma/beta, normalize -> xn (bf16)
            xn_bf = big.tile([128, CT, N], bf16)
            scs = []
            bc_ps = psM.tile([128, CT, 2], f32, tag="misc")  # [mean_c, rstd_c] per t
            for t in range(CT):
                nc.tensor.matmul(
                    bc_ps[:, t, :], sel_sb[:, t, :], stats_sb[:], start=True, stop=True
                )
            for t in range(CT):
                sc = work.tile([128, 2], f32, tag=f"gnsc{t}")  # [scale_c, bias_c]
                nc.vector.tensor_mul(sc[:, 0:1], gb_sb[:, t, 0:1], bc_ps[:, t, 1:2])
                nc.vector.tensor_mul(sc[:, 1:2], bc_ps[:, t, 0:1], sc[:, 0:1])
                nc.vector.tensor_tensor(
                    sc[:, 1:2], gb_sb[:, t, 1:2], sc[:, 1:2], ALU.subtract
                )
                scs.append(sc)

            # ---------------- QKV projections ----------------
            # Only K block 0 and Q block 0 are produced up front; the rest is
            # interleaved into the first attention segment so ScalarE starts
            # exp-ing as early as possible.
            k_bf = big.tile([128, N], bf16)      # [hid, keys] head-major
            q_bf = big.tile([128, NQ], bf16)     # [hid, queries] (pre-scaled W)
            v_bf = big.tile([128, NKB, 136], bf16)  # [key128, kb, 4x(32 V + 1 ones) + pad]

            def emit_k(cb):
                kp = psM.tile([128, 512], f32, tag="misc")
                for t in range(CT):
                    nc.tensor.matmul(
                        kp[:], wq_bf[:, t, HID : 2 * HID],
                        xn_bf[:, t, cb * 512 : (cb + 1) * 512],
                        start=(t == 0), stop=(t == CT - 1),
                    )
                if cb == 0:  # ScalarE is idle pre-attention; keep DVE free
                    nc.scalar.activation(
                        k_bf[:, 0:512], kp[:], AF.Identity
                    )
                else:
                    nc.vector.tensor_copy(k_bf[:, cb * 512 : (cb + 1) * 512], kp[:])

            def emit_q(cb):
                if cb == 0:  # borrow psO's bank: parallel with K(0) in psM
                    qp = psO.tile([128, 512], f32, tag="oacc", name="qp0")
                else:
                    qp = psM.tile([128, 512], f32, tag="misc")
                for t in range(CT):
                    nc.tensor.matmul(
                        qp[:], wq_bf[:, t, 0:HID],
                        xn_bf[:, t, cb * 512 : (cb + 1) * 512],
                        start=(t == 0), stop=(t == CT - 1),
                    )
                if cb == 0:
                    nc.scalar.activation(
                        q_bf[:, 0:512], qp[:], AF.Identity
                    )
                else:
                    nc.vector.tensor_copy(q_bf[:, cb * 512 : (cb + 1) * 512], qp[:])

            def emit_v(nb):
                vp = psM.tile([128, 128], f32, tag="misc")
                for t in range(CT):
                    nc.tensor.matmul(
                        vp[:], xn_bf[:, t, nb * 128 : (nb + 1) * 128],
                        wq_bf[:, t, 2 * HID : 3 * HID],
                        start=(t == 0), stop=(t == CT - 1),
                    )
                nc.vector.tensor_copy(
                    v_bf[:, nb, 0:132].rearrange("p (h c) -> p h c", c=33)[:, :, 0:32],
                    vp.rearrange("p (h c) -> p h c", c=32),
                )

            nc.vector.memset(
                v_bf[:, :, 0:132].rearrange("p k (h c) -> p k h c", c=33)[:, :, :, 32:33],
                1.0,
            )
            def emit_xn(j):
                c_sl = slice(j * CHK, (j + 1) * CHK)
                for t in range(CT):
                    nc.vector.tensor_scalar(
                        xn_bf[:, t, c_sl], x_sb[:, t, c_sl],
                        scalar1=scs[t][:, 0:1], scalar2=scs[t][:, 1:2],
                        op0=ALU.mult, op1=ALU.add,
                    )

            emit_xn(0)
            emit_k(0)
            emit_q(0)

            # ---------------- attention + out-projection ----------------
            h_bf = big.tile([128, NQ], bf16)     # [hid, queries], normalized
            ones32 = big.tile([1, 32], bf16)
            nc.vector.memset(ones32[:], 1.0)

            def make_normalize(qb, pair, oacc):
                # epilogue for one (qb, pair): 1/denominator, rank-1 bcast to
                # 32 partitions (bf16 matmul), scale O^T -> h_bf
                def _emit():
                    q_sl = slice(qb * 512, (qb + 1) * 512)
                    # one copy releases the psO slot quickly; the epilogue
                    # then works from SBUF
                    ocp = work.tile([128, 512], f32, tag="ocp")
                    nc.vector.tensor_copy(ocp[:], oacc[:])
                    for hi in range(2):
                        h = 2 * pair + hi
                        r = work.tile([1, 512], bf16, tag="r")
                        with nc.allow_low_precision(
                            reason="softmax denom reciprocal at bf16 matches matmul dtype"
                        ):
                            nc.vector.reciprocal(
                                r[:], ocp[64 * hi + 32 : 64 * hi + 33, :]
                            )
                        rb = psM.tile([32, 512], f32, tag="misc")
                        nc.tensor.matmul(rb[:], ones32[:], r[:], start=True, stop=True)
                        rbs = work.tile([128, 512], f32, tag="rbs")
                        nc.vector.tensor_copy(rbs[64 * hi : 64 * hi + 32, :], rb[:])
                        nc.vector.tensor_tensor(
                            h_bf[32 * h : 32 * h + 32, q_sl],
                            ocp[64 * hi : 64 * hi + 32, :],
                            rbs[64 * hi : 64 * hi + 32, :],
                            ALU.mult,
                        )
                return _emit

            def make_outproj(qb, use_act=False):
                def _emit():
                    q_sl = slice(qb * 512, (qb + 1) * 512)
                    for oc in range(CT):
                        yp = psM.tile([128, 512], f32, tag="misc")
                        nc.tensor.matmul(
                            yp[:], wo_bf[:, oc * 128 : (oc + 1) * 128], h_bf[:, q_sl],
                            start=True, stop=True,
                        )
                        yt = work.tile([128, 512], bf16, tag="yt")
                        if use_act:  # tail: ScalarE is idle after the last exp
                            nc.scalar.activation(
                                yt[:], yp[:], AF.Identity, bias=gb_sb[:, oc, 2:3]
                            )
                        else:
                            nc.vector.tensor_scalar(
                                yt[:], yp[:], scalar1=gb_sb[:, oc, 2:3], scalar2=None,
                                op0=ALU.add,
                            )
                        nc.sync.dma_start(out_d[oc, :, q_sl], yt[:])
                return _emit

            # per-segment slice index i in [0, 64): kb = i//2, head = pair*2 + i%2.
            # ScalarE exp chunks cover 3 slices (1536 elems/partition) to
            # amortize the per-ACTIVATE overhead; last two chunks are 2-wide.
            CH_SIZES = [2, 3, 3, 3, 3, 3, 3, 3, 3, 3, 3, 3, 3, 3, 3, 3, 3, 3, 3, 3, 3, 2]
            pending = []  # epilogues deferred so PE keeps feeding ACT first
            for qb in range(NQB):
                q_sl = slice(qb * 512, (qb + 1) * 512)
                for pair in range(2):
                    seg0 = qb == 0 and pair == 0
                    if pending:
                        for fn in pending:
                            fn()
                        pending = []
                    oacc = psO.tile([128, 512], f32, tag="oacc")
                    i = 0
                    for ci, ch in enumerate(CH_SIZES):
                        simp = psS.tile([128, ch, 512], f32, tag="sim")
                        for s in range(ch):
                            kb, hi = (i + s) // 2, (i + s) % 2
                            h = 2 * pair + hi
                            nc.tensor.matmul(
                                simp[:, s, :],
                                k_bf[32 * h : 32 * h + 32, kb * 128 : (kb + 1) * 128],
                                q_bf[32 * h : 32 * h + 32, q_sl],
                                start=True, stop=True,
                                tile_position=(32 * h, 0),
                            )
                        if seg0:
                            # produce the remaining xn chunks, V_T for this
                            # chunk's key blocks, and the next K column block,
                            # overlapped with the exp
                            if ci in (1, 6, 11):
                                emit_xn({1: 1, 6: 2, 11: 3}[ci])
                            for kb in range((i + 1) // 2, (i + ch + 1) // 2):
                                emit_v(kb)
                                if kb % 4 == 0 and 0 < kb // 4 + 1 < 8:
                                    emit_k(kb // 4 + 1)
                        if qb < NQB - 1 and pair == 1 and ci == 8:
                            emit_q(qb + 1)
                        e = epool.tile([128, ch, 512], bf16, tag="e")
                        nc.scalar.activation(e[:], simp[:], AF.Exp)
                        for s in range(ch):
                            kb, hi = (i + s) // 2, (i + s) % 2
                            h = 2 * pair + hi
                            nc.tensor.matmul(
                                oacc[64 * hi : 64 * hi + 33, :],
                                v_bf[:, kb, 33 * h : 33 * h + 33],
                                e[:, s, :],
                                start=(kb == 0), stop=(kb == NKB - 1),
                                tile_position=(0, 64 * hi),
                            )
                        i += ch
                    pending.append(make_normalize(qb, pair, oacc))
                    if pair == 1:
                        pending.append(make_outproj(qb, use_act=(qb == NQB - 1)))
            for fn in pending:
                fn()
    return nc


def _prep_shared(w_qkv, w_out, b_out, gamma, beta):
    scale = DH ** -0.5
    wqkvT = np.ascontiguousarray(w_qkv.T).astype(np.float32).copy()  # [C, 384]
    wqkvT[:, :HID] *= scale
    wq = np.ascontiguousarray(wqkvT.reshape(CT, 128, 3 * HID))
    wo = np.ascontiguousarray(w_out.T).astype(np.float32)            # [HID, C]
    gb = np.stack(
        [
            np.asarray(gamma, np.float32).reshape(CT, 128).T,
            np.asarray(beta, np.float32).reshape(CT, 128).T,
            np.asarray(b_out, np.float32).reshape(CT, 128).T,
        ],
        axis=-1,
    )  # [128, CT, 3]
    gmask = np.zeros((128, CT, G), np.float32)
    sel = np.zeros((G, CT, 128), np.float32)
    for t in range(CT):
        for p in range(128):
            g = (t * 128 + p) // (C // G)
            gmask[p, t, g] = 1.0 / (C // G)
            sel[g, t, p] = 1.0
    return wq, wo, gb, gmask, sel


def _run(inputs, trace=False):
    from concourse.bass_utils import run_bass_kernel_spmd

    x = np.asarray(inputs["x"], np.float32)
    wq, wo, gb, gmask, sel = _prep_shared(
        np.asarray(inputs["w_qkv"], np.float32),
        np.asarray(inputs["w_out"], np.float32),
        np.asarray(inputs["b_out"], np.float32),
        np.asarray(inputs["gamma"], np.float32),
        np.asarray(inputs["beta"], np.float32),
    )
    if "nc" not in _BUILT:
        _BUILT["nc"] = build_nc()
    nc = _BUILT["nc"]

    in_maps = []
    for core in range(8):
        b_idx, qh = core // 2, core % 2
        xb = x[b_idx].reshape(C, N)
        if qh:
            xb = np.roll(xb, -NQ, axis=1)
        in_maps.append(
            {
                "x": np.ascontiguousarray(
                    xb.reshape(CT, 128, N).astype(ml_dtypes.bfloat16)
                ),
                "wq": wq, "wo": wo, "gb": gb, "gmask": gmask, "selT": sel,
            }
        )
    res = run_bass_kernel_spmd(
        nc, in_maps, core_ids=list(range(8)), trace=trace
    )
    out = np.empty((B, C, N), np.float32)
    for core in range(8):
        b_idx, qh = core // 2, core % 2
        y = res.results[core]["out"].astype(np.float32).reshape(C, NQ)
        out[b_idx, :, qh * NQ : (qh + 1) * NQ] = y
    return out.reshape(B, C, HW, HW), res


def kernel(**inputs) -> np.ndarray:
    out, _ = _run(inputs, trace=False)
    return out


# revision 52
# speedup vs baseline: 1.0055x; 1.0031x over previous
"""Trainium2 Bass kernel: GroupNorm(8) -> 1x1 QKV conv -> 4-head attention
(n=4096, dim_head=32) -> 1x1 out conv, for x[4, 256, 64, 64] f32.

Sharding (8 cores, SPMD, no collectives): core c handles batch c//2 and query
half c%2. Each core receives the full 256x4096 (channels x spatial) slab of
its batch -- spatially ROLLED by 2048 for odd cores, so the program always
computes attention outputs for "queries 0:2048". GroupNorm statistics and
attention are invariant under a permutation of the key/spatial axis, so the
rolled copy yields exactly the outputs of the core's query half.

Device-side layout highlights:
  - sim is computed transposed, [keys, queries], per head at PE row-strip 32h
    (head pairs run concurrently via tile_position row tiling), so softmax's
    denominator folds into the attn@V matmul as an appended ones-column
    (stationary V_ext [128 keys, 33]) -> psum rows 0:32 hold O^T [dh, q]
    (exactly the layout the out-projection wants), row 32 the denominator.
  - exp runs on ScalarE from PSUM f32 -> SBUF bf16 in [128, 3x512] chunks
    (6 of the 8 psum banks double-buffer the sim tiles; 1 bank accumulates
    O^T; 1 bank serves projections/broadcasts). ScalarE is the wall-clock
    bottleneck: ~33.5M exps per core at ~1 elem/cycle/lane.
  - all matmuls are bf16 (fp32 matmul is 4x slower on TRN2); x ships as bf16
    to halve the startup HBM load; everything hidden under exp except the
    ~15us in-ramp and ~11us drain tail (cost-model timeline ~298us/core).
  - K/Q/V_T and the remaining GroupNorm-ed x chunks are produced inside the
    first attention segment, borrowing idle engine time under the exp stream;
    per-(qb,pair) epilogues (reciprocal + rank-1 broadcast matmul + scale,
    out-projection) are deferred past the next segment's first chunks so the
    PE keeps feeding ScalarE.
"""

import ml_dtypes
import numpy as np

HEADS, DH, G, EPS = 4, 32, 8, 1e-5
B, C, HW = 4, 256, 64
N = HW * HW          # 4096 spatial positions (keys)
NQ = N // 2          # 2048 queries per core
NKB = N // 128       # 32 key blocks
NQB = NQ // 512      # 4 query blocks of 512
CT = C // 128        # 2 channel tiles
HID = HEADS * DH     # 128

_BUILT = {}


_MAX_INST_WAITS = 1


def _patch_tail_drain(tile_mod):
    """Walrus codegen on this toolchain only supports a small number of sync
    waits per ISA instruction. Two patches:
    1. every committed instruction with too many waits gets the excess hoisted
       onto same-engine nops emitted immediately before it (same stream
       position => identical semantics);
    2. the TileContext tail drain (one wait per engine + DMA lane, >8 total)
       is split one wait per SP nop."""
    if getattr(tile_mod.TileContext, "_drain_patched", False):
        return
    import bass_rust
    from concourse.vector_clock import ScopedClock

    _orig_add_instruction = tile_mod.TileContext._add_instruction

    _SELF_WAIT_OK = ("InstActivation",)

    def _add_with_wait_split(self, inst):
        si = getattr(inst, "sync_info", None)
        if (
            si is not None
            and si.on_wait
            and type(inst).__name__ in _SELF_WAIT_OK
        ):
            # engine queues are strict FIFO: a data op's wait on its own
            # engine's sem is redundant (prior same-engine ops complete in
            # order before it) -- dropping it avoids a split nop per op
            eng_name = str(inst.engine).split(".")[-1]
            kept = [
                w for w in si.on_wait
                if w.ant_name.rsplit("_", 1)[0] != eng_name
            ]
            if len(kept) != len(si.on_wait):
                import bass_rust as _br
                inst.sync_info = _br.SyncInfo(
                    on_wait=kept, on_update=list(si.on_update)
                )
                si = inst.sync_info
        if si is not None and len(si.on_wait) > _MAX_INST_WAITS:
            waits = list(si.on_wait)
            keep, excess = waits[: _MAX_INST_WAITS], waits[_MAX_INST_WAITS :]
            eng = self.nc.engines[inst.engine]
            for w in excess:  # NoOps only support a single wait slot
                nop = eng.nop(nofuse=True, hint="wait_split")
                nop.ins.sync_info = bass_rust.SyncInfo(on_wait=[w], on_update=[])
            inst.sync_info = bass_rust.SyncInfo(
                on_wait=keep, on_update=list(si.on_update)
            )
        return _orig_add_instruction(self, inst)

    tile_mod.TileContext._add_instruction = _add_with_wait_split

    def _patched(self, tick_clock, wait_clock):
        nop = self.nc.sync.nop(nofuse=True, hint="pre_drain_wait_split")
        wait_clock.add_sem_waits(nop.ins, ScopedClock({None: tick_clock.global_clock}))
        si = nop.ins.sync_info
        waits = list(si.on_wait) if si is not None else []
        if len(waits) > 1:
            nop.ins.sync_info = bass_rust.SyncInfo(
                on_wait=waits[:1], on_update=list(si.on_update)
            )
            for i in range(1, len(waits)):
                n2 = self.nc.sync.nop(nofuse=True, hint="pre_drain_wait_split")
                n2.ins.sync_info = bass_rust.SyncInfo(on_wait=[waits[i]], on_update=[])
        self.nc.sync.drain()
        self.nc.all_engine_barrier()
        assert self.sems is not None
        popped = self.nc._tile_sem_poison_stack.pop()
        assert popped is self._sem_poison
        self.nc.clear_and_free_semaphores(list(self.sems.allocated().values()))
        # no trailing all_engine_barrier: NEFF completion already requires the
        # SP stream (which performs the sem clears) to finish, and every other
        # engine is quiesced by the barrier above

    tile_mod.TileContext._drain_and_barrier = _patched
    tile_mod.TileContext._drain_patched = True


def build_nc():
    import concourse.bass as bass
    import concourse.mybir as mybir
    import concourse.tile as tile

    _patch_tail_drain(tile)
    f32 = mybir.dt.float32
    bf16 = mybir.dt.bfloat16
    AF = mybir.ActivationFunctionType
    ALU = mybir.AluOpType

    nc = bass.Bass()
    x_d = nc.declare_dram_parameter("x", [CT, 128, N], bf16, isOutput=False)
    wq_d = nc.declare_dram_parameter("wq", [CT, 128, 3 * HID], f32, isOutput=False)
    wo_d = nc.declare_dram_parameter("wo", [HID, C], f32, isOutput=False)
    gb_d = nc.declare_dram_parameter("gb", [128, CT, 3], f32, isOutput=False)
    gmask_d = nc.declare_dram_parameter("gmask", [128, CT, G], f32, isOutput=False)
    sel_d = nc.declare_dram_parameter("selT", [G, CT, 128], f32, isOutput=False)
    out_d = nc.declare_dram_parameter("out", [CT, 128, NQ], bf16, isOutput=True)

    with tile.TileContext(nc) as tc:
        with (
            tc.tile_pool(name="big", bufs=1) as big,
            tc.tile_pool(name="work", bufs=2) as work,
            tc.tile_pool(name="epool", bufs=3) as epool,
            tc.tile_pool(name="psS", bufs=2, space="PSUM") as psS,  # 2 x 3 banks
            tc.tile_pool(name="psO", bufs=1, space="PSUM") as psO,  # 1 bank
            tc.tile_pool(name="psM", bufs=1, space="PSUM") as psM,  # 1 bank
        ):
            # ---------------- load inputs ----------------
            x_sb = big.tile([128, CT, N], bf16)
            for t in range(CT):
                for j in range(4):
                    nc.sync.dma_start(
                        x_sb[:, t, j * 1024 : (j + 1) * 1024],
                        x_d[t, :, j * 1024 : (j + 1) * 1024],
                    )
            wq_sb = big.tile([128, CT, 3 * HID], f32)
            nc.sync.dma_start(wq_sb[:, 0], wq_d[0])
            nc.sync.dma_start(wq_sb[:, 1], wq_d[1])
            wo_sb = big.tile([HID, C], f32)
            nc.sync.dma_start(wo_sb[:], wo_d[:])
            gb_sb = big.tile([128, CT, 3], f32)
            nc.sync.dma_start(gb_sb[:], gb_d[:])
            gmask_sb = big.tile([128, CT, G], f32)
            nc.sync.dma_start(gmask_sb[:], gmask_d[:])
            sel_sb = big.tile([G, CT, 128], f32)
            nc.sync.dma_start(sel_sb[:], sel_d[:])

            wq_bf = big.tile([128, CT, 3 * HID], bf16)
            nc.scalar.activation(wq_bf[:], wq_sb[:], AF.Identity)
            wo_bf = big.tile([HID, C], bf16)
            nc.scalar.activation(wo_bf[:], wo_sb[:], AF.Identity)

            # ---------------- group norm ----------------
            # per-channel mean/var via DVE bn_stats (512-col chunks, overlaps
            # the x DMAs); T = [mean_c, E[x^2]_c]
            NJ = 4
            CHK = N // NJ
            T_sb = work.tile([128, CT, 2], f32, tag="gnT")
            for t in range(CT):
                bst = work.tile([128, 8, nc.vector.BN_STATS_DIM], f32, tag="gnbst")
                for j in range(8):
                    nc.vector.bn_stats(
                        out=bst[:, j, :], in_=x_sb[:, t, j * 512 : (j + 1) * 512]
                    )
                mv = work.tile([128, nc.vector.BN_AGGR_DIM], f32, tag="gnmv")
                nc.vector.bn_aggr(out=mv[:], in_=bst[:])
                # T[:,t] = [mean, var + mean^2]
                nc.vector.tensor_copy(T_sb[:, t, 0:1], mv[:, 0:1])
                nc.vector.tensor_mul(T_sb[:, t, 1:2], mv[:, 0:1], mv[:, 0:1])
                nc.vector.tensor_tensor(
                    T_sb[:, t, 1:2], T_sb[:, t, 1:2], mv[:, 1:2], ALU.add
                )
            # group stats: [G, 2] = sum over channels-in-group / (32*4096)
            stats_ps = psM.tile([G, 2], f32, tag="misc")
            for t in range(CT):
                nc.tensor.matmul(
                    stats_ps[:], gmask_sb[:, t, :], T_sb[:, t, :],
                    start=(t == 0), stop=(t == CT - 1),
                )
            # var = E[x^2] - mean^2 ; rstd = 1/sqrt(var+eps)
            stats_sb = work.tile([G, 2], f32, tag="gnstats")  # [mean, rstd]
            stats_tmp = work.tile([G, 2], f32, tag="gnstats_raw")
            msq = work.tile([G, 1], f32, tag="gnmsq")
            nc.vector.tensor_copy(stats_tmp[:], stats_ps[:])
            nc.vector.tensor_copy(stats_sb[:, 0:1], stats_tmp[:, 0:1])
            nc.vector.tensor_mul(msq[:], stats_tmp[:, 0:1], stats_tmp[:, 0:1])
            nc.vector.tensor_tensor(
                stats_sb[:, 1:2], stats_tmp[:, 1:2], msq[:], ALU.subtract
            )
            eps_sb = work.tile([G, 1], f32, tag="gneps")
            nc.vector.memset(eps_sb[:], EPS)
            nc.scalar.activation(
                stats_sb[:, 1:2], stats_sb[:, 1:2], AF.Sqrt, bias=eps_sb[:]
            )
            nc.vector.reciprocal(stats_sb[:, 1:2], stats_sb[:, 1:2])
            # broadcast to channels, fold gamma/beta, normalize -> xn (bf16)
            xn_bf = big.tile([128, CT, N], bf16)
            scs = []
            bc_ps = psM.tile([128, CT, 2], f32, tag="misc")  # [mean_c, rstd_c] per t
            for t in range(CT):
                nc.tensor.matmul(
                    bc_ps[:, t, :], sel_sb[:, t, :], stats_sb[:], start=True, stop=True
                )
            for t in range(CT):
                sc = work.tile([128, 2], f32, tag=f"gnsc{t}")  # [scale_c, bias_c]
                nc.vector.tensor_mul(sc[:, 0:1], gb_sb[:, t, 0:1], bc_ps[:, t, 1:2])
                nc.vector.tensor_mul(sc[:, 1:2], bc_ps[:, t, 0:1], sc[:, 0:1])
                nc.vector.tensor_tensor(
                    sc[:, 1:2], gb_sb[:, t, 1:2], sc[:, 1:2], ALU.subtract
                )
                scs.append(sc)

            # ---------------- QKV projections ----------------
            # Only K block 0 and Q block 0 are produced up front; the rest is
            # interleaved into the first attention segment so ScalarE starts
            # exp-ing as early as possible.
            k_bf = big.tile([128, N], bf16)      # [hid, keys] head-major
            q_bf = big.tile([128, NQ], bf16)     # [hid, queries] (pre-scaled W)
            v_bf = big.tile([128, NKB, 136], bf16)  # [key128, kb, 4x(32 V + 1 ones) + pad]

            def emit_k(cb):
                kp = psM.tile([128, 512], f32, tag="misc")
                for t in range(CT):
                    nc.tensor.matmul(
                        kp[:], wq_bf[:, t, HID : 2 * HID],
                        xn_bf[:, t, cb * 512 : (cb + 1) * 512],
                        start=(t == 0), stop=(t == CT - 1),
                    )
                if cb == 0:  # ScalarE is idle pre-attention; keep DVE free
                    nc.scalar.activation(
                        k_bf[:, 0:512], kp[:], AF.Identity
                    )
                else:
                    nc.vector.tensor_copy(k_bf[:, cb * 512 : (cb + 1) * 512], kp[:])

            def emit_q(cb):
                if cb == 0:  # borrow psO's bank: parallel with K(0) in psM
                    qp = psO.tile([128, 512], f32, tag="oacc", name="qp0")
                else:
                    qp = psM.tile([128, 512], f32, tag="misc")
                for t in range(CT):
                    nc.tensor.matmul(
                        qp[:], wq_bf[:, t, 0:HID],
                        xn_bf[:, t, cb * 512 : (cb + 1) * 512],
                        start=(t == 0), stop=(t == CT - 1),
                    )
                if cb == 0:
                    nc.scalar.activation(
                        q_bf[:, 0:512], qp[:], AF.Identity
                    )
                else:
                    nc.vector.tensor_copy(q_bf[:, cb * 512 : (cb + 1) * 512], qp[:])

            def emit_v(nb):
                vp = psM.tile([128, 128], f32, tag="misc")
                for t in range(CT):
                    nc.tensor.matmul(
                        vp[:], xn_bf[:, t, nb * 128 : (nb + 1) * 128],
                        wq_bf[:, t, 2 * HID : 3 * HID],
                        start=(t == 0), stop=(t == CT - 1),
                    )
                nc.vector.tensor_copy(
                    v_bf[:, nb, 0:132].rearrange("p (h c) -> p h c", c=33)[:, :, 0:32],
                    vp.rearrange("p (h c) -> p h c", c=32),
                )

            nc.vector.memset(
                v_bf[:, :, 0:132].rearrange("p k (h c) -> p k h c", c=33)[:, :, :, 32:33],
                1.0,
            )
            def emit_xn(j):
                c_sl = slice(j * CHK, (j + 1) * CHK)
                for t in range(CT):
                    nc.vector.tensor_scalar(
                        xn_bf[:, t, c_sl], x_sb[:, t, c_sl],
                        scalar1=scs[t][:, 0:1], scalar2=scs[t][:, 1:2],
                        op0=ALU.mult, op1=ALU.add,
                    )

            emit_xn(0)
            emit_k(0)
            emit_q(0)

            # ---------------- attention + out-projection ----------------
            h_bf = big.tile([128, NQ], bf16)     # [hid, queries], normalized
            ones32 = big.tile([1, 32], bf16)
            nc.vector.memset(ones32[:], 1.0)

            def make_normalize(qb, pair, oacc):
                # epilogue for one (qb, pair): 1/denominator, rank-1 bcast to
                # 32 partitions (bf16 matmul), scale O^T -> h_bf
                def _emit():
                    q_sl = slice(qb * 512, (qb + 1) * 512)
                    # one copy releases the psO slot quickly; the epilogue
                    # then works from SBUF
                    ocp = work.tile([128, 512], f32, tag="ocp")
                    nc.vector.tensor_copy(ocp[:], oacc[:])
                    for hi in range(2):
                        h = 2 * pair + hi
                        r = work.tile([1, 512], bf16, tag="r")
                        with nc.allow_low_precision(
                            reason="softmax denom reciprocal at bf16 matches matmul dtype"
                        ):
                            nc.vector.reciprocal(
                                r[:], ocp[64 * hi + 32 : 64 * hi + 33, :]
                            )
                        rb = psM.tile([32, 512], f32, tag="misc")
                        nc.tensor.matmul(rb[:], ones32[:], r[:], start=True, stop=True)
                        rbs = work.tile([128, 512], f32, tag="rbs")
                        nc.vector.tensor_copy(rbs[64 * hi : 64 * hi + 32, :], rb[:])
                        nc.vector.tensor_tensor(
                            h_bf[32 * h : 32 * h + 32, q_sl],
                            ocp[64 * hi : 64 * hi + 32, :],
                            rbs[64 * hi : 64 * hi + 32, :],
                            ALU.mult,
                        )
                return _emit

            def make_outproj(qb, use_act=False):
                def _emit():
                    q_sl = slice(qb * 512, (qb + 1) * 512)
                    for oc in range(CT):
                        yp = psM.tile([128, 512], f32, tag="misc")
                        nc.tensor.matmul(
                            yp[:], wo_bf[:, oc * 128 : (oc + 1) * 128], h_bf[:, q_sl],
                            start=True, stop=True,
                        )
                        yt = work.tile([128, 512], bf16, tag="yt")
                        if use_act:  # tail: ScalarE is idle after the last exp
                            nc.scalar.activation(
                                yt[:], yp[:], AF.Identity, bias=gb_sb[:, oc, 2:3]
                            )
                        else:
                            nc.vector.tensor_scalar(
                                yt[:], yp[:], scalar1=gb_sb[:, oc, 2:3], scalar2=None,
                                op0=ALU.add,
                            )
                        nc.sync.dma_start(out_d[oc, :, q_sl], yt[:])
                return _emit

            # per-segment slice index i in [0, 64): kb = i//2, head = pair*2 + i%2.
            # ScalarE exp chunks cover 3 slices (1536 elems/partition) to
            # amortize the per-ACTIVATE overhead; last two chunks are 2-wide.
            CH_SIZES = [2, 3, 3, 3, 3, 3, 3, 3, 3, 3, 3, 3, 3, 3, 3, 3, 3, 3, 3, 3, 3, 2]
            pending = []  # epilogues deferred so PE keeps feeding ACT first
            for qb in range(NQB):
                q_sl = slice(qb * 512, (qb + 1) * 512)
                for pair in range(2):
                    seg0 = qb == 0 and pair == 0
                    if pending:
                        for fn in pending:
                            fn()
                        pending = []
                    oacc = psO.tile([128, 512], f32, tag="oacc")
                    i = 0
                    for ci, ch in enumerate(CH_SIZES):
                        simp = psS.tile([128, ch, 512], f32, tag="sim")
                        for s in range(ch):
                            kb, hi = (i + s) // 2, (i + s) % 2
                            h = 2 * pair + hi
                            nc.tensor.matmul(
                                simp[:, s, :],
                                k_bf[32 * h : 32 * h + 32, kb * 128 : (kb + 1) * 128],
                                q_bf[32 * h : 32 * h + 32, q_sl],
                                start=True, stop=True,
                                tile_position=(32 * h, 0),
                            )
                        if seg0:
                            # produce the remaining xn chunks, V_T for this
                            # chunk's key blocks, and the next K column block,
                            # overlapped with the exp
                            if ci in (1, 6, 11):
                                emit_xn({1: 1, 6: 2, 11: 3}[ci])
                            for kb in range((i + 1) // 2, (i + ch + 1) // 2):
                                emit_v(kb)
                                if kb % 4 == 0 and 0 < kb // 4 + 1 < 8:
                                    emit_k(kb // 4 + 1)
                        if qb < NQB - 1 and pair == 1 and ci == 8:
                            emit_q(qb + 1)
                        e = epool.tile([128, ch, 512], bf16, tag="e")
                        nc.scalar.activation(e[:], simp[:], AF.Exp)
                        for s in range(ch):
                            kb, hi = (i + s) // 2, (i + s) % 2
                            h = 2 * pair + hi
                            nc.tensor.matmul(
                                oacc[64 * hi : 64 * hi + 33, :],
                                v_bf[:, kb, 33 * h : 33 * h + 33],
                                e[:, s, :],
                                start=(kb == 0), stop=(kb == NKB - 1),
                                tile_position=(0, 64 * hi),
                            )
                        i += ch
                    pending.append(make_normalize(qb, pair, oacc))
                    if pair == 1:
                        pending.append(make_outproj(qb, use_act=(qb == NQB - 1)))
            for fn in pending:
                fn()
    return nc


def _prep_shared(w_qkv, w_out, b_out, gamma, beta):
    scale = DH ** -0.5
    wqkvT = np.ascontiguousarray(w_qkv.T).astype(np.float32).copy()  # [C, 384]
    wqkvT[:, :HID] *= scale
    wq = np.ascontiguousarray(wqkvT.reshape(CT, 128, 3 * HID))
    wo = np.ascontiguousarray(w_out.T).astype(np.float32)            # [HID, C]
    gb = np.stack(
        [
            np.asarray(gamma, np.float32).reshape(CT, 128).T,
            np.asarray(beta, np.float32).reshape(CT, 128).T,
            np.asarray(b_out, np.float32).reshape(CT, 128).T,
        ],
        axis=-1,
    )  # [128, CT, 3]
    gmask = np.zeros((128, CT, G), np.float32)
    sel = np.zeros((G, CT, 128), np.float32)
    for t in range(CT):
        for p in range(128):
            g = (t * 128 + p) // (C // G)
            gmask[p, t, g] = 1.0 / (C // G)
            sel[g, t, p] = 1.0
    return wq, wo, gb, gmask, sel


def _run(inputs, trace=False):
    from concourse.bass_utils import run_bass_kernel_spmd

    x = np.asarray(inputs["x"], np.float32)
    wq, wo, gb, gmask, sel = _prep_shared(
        np.asarray(inputs["w_qkv"], np.float32),
        np.asarray(inputs["w_out"], np.float32),
        np.asarray(inputs["b_out"], np.float32),
        np.asarray(inputs["gamma"], np.float32),
        np.asarray(inputs["beta"], np.float32),
    )
    if "nc" not in _BUILT:
        _BUILT["nc"] = build_nc()
    nc = _BUILT["nc"]

    in_maps = []
    for core in range(8):
        b_idx, qh = core // 2, core % 2
        xb = x[b_idx].reshape(C, N)
        if qh:
            xb = np.roll(xb, -NQ, axis=1)
        in_maps.append(
            {
                "x": np.ascontiguousarray(
                    xb.reshape(CT, 128, N).astype(ml_dtypes.bfloat16)
                ),
                "wq": wq, "wo": wo, "gb": gb, "gmask": gmask, "selT": sel,
            }
        )
    res = run_bass_kernel_spmd(
        nc, in_maps, core_ids=list(range(8)), trace=trace
    )
    out = np.empty((B, C, N), np.float32)
    for core in range(8):
        b_idx, qh = core // 2, core % 2
        y = res.results[core]["out"].astype(np.float32).reshape(C, NQ)
        out[b_idx, :, qh * NQ : (qh + 1) * NQ] = y
    return out.reshape(B, C, HW, HW), res


def kernel(**inputs) -> np.ndarray:
    out, _ = _run(inputs, trace=False)
    return out
